# revision 2
# baseline (speedup 1.0000x reference)
"""Trainium2 Bass kernel for sliding-window unfold (im2col).

reference:  out = x[:, idx, :]  with idx[w, f] = w + f
  x:   [128, 4096, 4]  f32
  out: [128, 4065, 32, 4]  f32

Key structural fact: out[b, w] (= 32*4 = 128 floats = 512 B) is the
contiguous slice x[b].flat[128*w : 128*w + 128].  The whole problem is a
sliding-window byte replication; HBM write bandwidth is the roofline.

Strategy (pure data parallel, batch 128 -> 16 per core on 8 cores):
per batch b on each core:
  1. one DMA loads a replicated tile X[127 partitions, 252 floats]:
     partition p holds x[b].flat[128p : 128p+252] (rows 32p .. 32p+62),
     i.e. everything windows 32p..32p+31 touch.
  2. one DVE copy expands X -> Y[127, 4096] with an overlapping-stride
     read AP: Y[p, 128j+i] = X[p, 4j+i]  ->  partition p now holds
     windows 32p..32p+31 materialized contiguously (16 KB).
  3. one DMA stores Y[0:127, :] to out[b] windows 0..4063 - contiguous
     16 KB runs per partition, i.e. full-rate descriptors.
The 16 remaining tail windows (w = 4064, one per batch) are two tiny
DMAs for the whole core.
"""

import numpy as np

from concourse import bacc, mybir, tile
from concourse.bass_utils import run_bass_kernel_spmd

N_CORES = 8
B_FULL = 128
B = B_FULL // N_CORES  # 16 batches per core
S = 4096
C = 4
F = 32
W = S - F + 1   # 4065
FL = F * C      # 128 floats per window
XB = S * C      # 16384 floats per batch of x
OB = W * FL     # 520320 floats per batch of out

_cache = {}


def build_nc():
    nc = bacc.Bacc("TRN2", target_bir_lowering=False)
    x = nc.dram_tensor("x", [B, S, C], mybir.dt.float32, kind="ExternalInput")
    out = nc.dram_tensor("out", [B, W, F, C], mybir.dt.float32, kind="ExternalOutput")

    with tile.TileContext(nc) as tc:
        with (
            tc.tile_pool(name="xp", bufs=3) as xp,
            tc.tile_pool(name="yp", bufs=3) as yp,
            tc.tile_pool(name="tp", bufs=1) as tp,
        ):
            for b in range(B):
                X = xp.tile([128, 252], mybir.dt.float32)
                src = x[:].copy()
                src.ap = mybir.VecI64Pair([[128, 127], [1, 252]])
                src.offset = b * XB
                nc.sync.dma_start(out=X[0:127, 0:252], in_=src)

                Y = yp.tile([128, 4096], mybir.dt.float32)
                src2 = X[:].copy()
                src2.ap = mybir.VecI64Pair([[252, 127], [4, 32], [1, 128]])
                src2.offset = 0
                dst2 = Y[:].copy()
                dst2.ap = mybir.VecI64Pair([[4096, 127], [128, 32], [1, 128]])
                dst2.offset = 0
                nc.vector.tensor_copy(out=dst2, in_=src2)

                dst3 = out[:].copy()
                dst3.ap = mybir.VecI64Pair([[4096, 127], [1, 4096]])
                dst3.offset = b * OB
                nc.sync.dma_start(out=dst3, in_=Y[0:127, 0:4096])

            # tail: window 4064 of each batch = last 128 floats of x[b]
            T = tp.tile([16, 128], mybir.dt.float32)
            srcT = x[:].copy()
            srcT.ap = mybir.VecI64Pair([[XB, 16], [1, 128]])
            srcT.offset = XB - FL
            nc.sync.dma_start(out=T[0:16, 0:128], in_=srcT)
            dstT = out[:].copy()
            dstT.ap = mybir.VecI64Pair([[OB, 16], [1, 128]])
            dstT.offset = (W - 1) * FL
            nc.sync.dma_start(out=dstT, in_=T[0:16, 0:128])
    nc.finalize()
    return nc


def run_sharded(x: np.ndarray, trace: bool = False):
    """Shard batch across 8 cores, run, gather. Returns (out, raw results)."""
    if "nc" not in _cache:
        _cache["nc"] = build_nc()
    nc = _cache["nc"]

    x = np.ascontiguousarray(x, dtype=np.float32)
    in_maps = [{"x": x[i * B : (i + 1) * B]} for i in range(N_CORES)]
    res = run_bass_kernel_spmd(nc, in_maps, list(range(N_CORES)), trace=trace)
    out = np.concatenate([res.results[i]["out"] for i in range(N_CORES)], axis=0)
    return out, res


def kernel(x: np.ndarray) -> np.ndarray:
    out, _ = run_sharded(x, trace=False)
    return out


# revision 3
# speedup vs baseline: 2.1083x; 2.1083x over previous
"""Trainium2 Bass kernel for sliding-window unfold (im2col).

reference:  out = x[:, idx, :]  with idx[w, f] = w + f
  x:   [128, 4096, 4]  f32
  out: [128, 4065, 32, 4]  f32

Key structural fact: out[b, w] (= 32*4 = 128 floats = 512 B) is the
contiguous slice x[b].flat[128*w : 128*w + 128].  The whole problem is a
sliding-window byte replication; HBM write bandwidth is the roofline.

Strategy (pure data parallel, batch 128 -> 16 per core on 8 cores):
per batch b on each core:
  1. one DMA loads a replicated tile X[127 partitions, 252 floats]:
     partition p holds x[b].flat[128p : 128p+252] (rows 32p .. 32p+62),
     i.e. everything windows 32p..32p+31 touch.
  2. one DVE copy expands X -> Y[127, 4096] with an overlapping-stride
     read AP: Y[p, 128j+i] = X[p, 4j+i]  ->  partition p now holds
     windows 32p..32p+31 materialized contiguously (16 KB).
  3. one DMA stores Y[0:127, :] to out[b] windows 0..4063 - contiguous
     16 KB runs per partition, i.e. full-rate descriptors.
The 16 remaining tail windows (w = 4064, one per batch) are two tiny
DMAs for the whole core.
"""

import numpy as np

from concourse import bacc, mybir, tile
from concourse.bass_utils import run_bass_kernel_spmd

N_CORES = 8
B_FULL = 128
B = B_FULL // N_CORES  # 16 batches per core
S = 4096
C = 4
F = 32
W = S - F + 1   # 4065
FL = F * C      # 128 floats per window
XB = S * C      # 16384 floats per batch of x
OB = W * FL     # 520320 floats per batch of out

_cache = {}


def build_nc():
    nc = bacc.Bacc("TRN2", target_bir_lowering=False)
    x = nc.dram_tensor("x", [B, S, C], mybir.dt.float32, kind="ExternalInput")
    out = nc.dram_tensor("out", [B, W, F, C], mybir.dt.float32, kind="ExternalOutput")

    with tile.TileContext(nc) as tc:
        with (
            tc.tile_pool(name="xp", bufs=3) as xp,
            tc.tile_pool(name="yp", bufs=3) as yp,
            tc.tile_pool(name="tp", bufs=1) as tp,
        ):
            for b in range(B):
                X = xp.tile([128, 252], mybir.dt.float32)
                src = x[:].copy()
                src.ap = mybir.VecI64Pair([[128, 127], [1, 252]])
                src.offset = b * XB
                nc.gpsimd.dma_start(out=X[0:127, 0:252], in_=src)

                Y = yp.tile([128, 4096], mybir.dt.float32)
                src2 = X[:].copy()
                src2.ap = mybir.VecI64Pair([[252, 127], [4, 32], [1, 128]])
                src2.offset = 0
                dst2 = Y[:].copy()
                dst2.ap = mybir.VecI64Pair([[4096, 127], [128, 32], [1, 128]])
                dst2.offset = 0
                nc.vector.tensor_copy(out=dst2, in_=src2)

                dst3 = out[:].copy()
                dst3.ap = mybir.VecI64Pair([[4096, 127], [1, 4096]])
                dst3.offset = b * OB
                nc.gpsimd.dma_start(out=dst3, in_=Y[0:127, 0:4096])

            # tail: window 4064 of each batch = last 128 floats of x[b]
            T = tp.tile([16, 128], mybir.dt.float32)
            srcT = x[:].copy()
            srcT.ap = mybir.VecI64Pair([[XB, 16], [1, 128]])
            srcT.offset = XB - FL
            nc.sync.dma_start(out=T[0:16, 0:128], in_=srcT)
            dstT = out[:].copy()
            dstT.ap = mybir.VecI64Pair([[OB, 16], [1, 128]])
            dstT.offset = (W - 1) * FL
            nc.sync.dma_start(out=dstT, in_=T[0:16, 0:128])
    nc.finalize()
    return nc


def run_sharded(x: np.ndarray, trace: bool = False):
    """Shard batch across 8 cores, run, gather. Returns (out, raw results)."""
    if "nc" not in _cache:
        _cache["nc"] = build_nc()
    nc = _cache["nc"]

    x = np.ascontiguousarray(x, dtype=np.float32)
    in_maps = [{"x": x[i * B : (i + 1) * B]} for i in range(N_CORES)]
    res = run_bass_kernel_spmd(nc, in_maps, list(range(N_CORES)), trace=trace)
    out = np.concatenate([res.results[i]["out"] for i in range(N_CORES)], axis=0)
    return out, res


def kernel(x: np.ndarray) -> np.ndarray:
    out, _ = run_sharded(x, trace=False)
    return out


# revision 4
# speedup vs baseline: 3.2331x; 1.5335x over previous
"""Trainium2 Bass kernel for sliding-window unfold (im2col).

reference:  out = x[:, idx, :]  with idx[w, f] = w + f
  x:   [128, 4096, 4]  f32
  out: [128, 4065, 32, 4]  f32

Key structural fact: out[b, w] (= 32*4 = 128 floats = 512 B) is the
contiguous slice x[b].flat[128*w : 128*w + 128].  The whole problem is a
sliding-window byte replication; HBM write bandwidth is the roofline.

Strategy (pure data parallel, batch 128 -> 16 per core on 8 cores):
per batch b on each core:
  1. one DMA loads a replicated tile X[127 partitions, 252 floats]:
     partition p holds x[b].flat[128p : 128p+252] (rows 32p .. 32p+62),
     i.e. everything windows 32p..32p+31 touch.
  2. one DVE copy expands X -> Y[127, 4096] with an overlapping-stride
     read AP: Y[p, 128j+i] = X[p, 4j+i]  ->  partition p now holds
     windows 32p..32p+31 materialized contiguously (16 KB).
  3. one DMA stores Y[0:127, :] to out[b] windows 0..4063 - contiguous
     16 KB runs per partition, i.e. full-rate descriptors.
The 16 remaining tail windows (w = 4064, one per batch) are two tiny
DMAs for the whole core.
"""

import numpy as np

from concourse import bacc, mybir, tile
from concourse.bass_utils import run_bass_kernel_spmd

N_CORES = 8
B_FULL = 128
B = B_FULL // N_CORES  # 16 batches per core
S = 4096
C = 4
F = 32
W = S - F + 1   # 4065
FL = F * C      # 128 floats per window
XB = S * C      # 16384 floats per batch of x
OB = W * FL     # 520320 floats per batch of out

_cache = {}


def build_nc():
    nc = bacc.Bacc("TRN2", target_bir_lowering=False)
    x = nc.dram_tensor("x", [B, S, C], mybir.dt.float32, kind="ExternalInput")
    out = nc.dram_tensor("out", [B, W, F, C], mybir.dt.float32, kind="ExternalOutput")

    with tile.TileContext(nc) as tc:
        with (
            tc.tile_pool(name="xp", bufs=8) as xp,
            tc.tile_pool(name="yp", bufs=8) as yp,
            tc.tile_pool(name="tp", bufs=1) as tp,
        ):
            for b in range(B):
                X = xp.tile([128, 252], mybir.dt.float32)
                src = x[:].copy()
                src.ap = mybir.VecI64Pair([[128, 127], [1, 252]])
                src.offset = b * XB
                nc.gpsimd.dma_start(out=X[0:127, 0:252], in_=src)

                Y = yp.tile([128, 4096], mybir.dt.float32)
                src2 = X[:].copy()
                src2.ap = mybir.VecI64Pair([[252, 127], [4, 32], [1, 128]])
                src2.offset = 0
                dst2 = Y[:].copy()
                dst2.ap = mybir.VecI64Pair([[4096, 127], [128, 32], [1, 128]])
                dst2.offset = 0
                nc.vector.tensor_copy(out=dst2, in_=src2)

                dst3 = out[:].copy()
                dst3.ap = mybir.VecI64Pair([[4096, 127], [1, 4096]])
                dst3.offset = b * OB
                nc.gpsimd.dma_start(out=dst3, in_=Y[0:127, 0:4096])

            # tail: window 4064 of each batch = last 128 floats of x[b]
            T = tp.tile([16, 128], mybir.dt.float32)
            srcT = x[:].copy()
            srcT.ap = mybir.VecI64Pair([[XB, 16], [1, 128]])
            srcT.offset = XB - FL
            nc.sync.dma_start(out=T[0:16, 0:128], in_=srcT)
            dstT = out[:].copy()
            dstT.ap = mybir.VecI64Pair([[OB, 16], [1, 128]])
            dstT.offset = (W - 1) * FL
            nc.sync.dma_start(out=dstT, in_=T[0:16, 0:128])
    nc.finalize()
    return nc


def run_sharded(x: np.ndarray, trace: bool = False):
    """Shard batch across 8 cores, run, gather. Returns (out, raw results)."""
    if "nc" not in _cache:
        _cache["nc"] = build_nc()
    nc = _cache["nc"]

    x = np.ascontiguousarray(x, dtype=np.float32)
    in_maps = [{"x": x[i * B : (i + 1) * B]} for i in range(N_CORES)]
    res = run_bass_kernel_spmd(nc, in_maps, list(range(N_CORES)), trace=trace)
    out = np.concatenate([res.results[i]["out"] for i in range(N_CORES)], axis=0)
    return out, res


def kernel(x: np.ndarray) -> np.ndarray:
    out, _ = run_sharded(x, trace=False)
    return out
